# revision 29
# baseline (speedup 1.0000x reference)
"""Trainium2 Bass kernel for ErosionP4 (P4 group-equivariant grayscale erosion).

Reference computation (shapes hardcoded):
  x: [B=4, G=4, H=96, W=96, C=4] fp32, kernel: [5, 5, 3, C=4, F=8] fp32
  out[b,g,h,w,f] = sum_c min_{k,dy,dx} ( ygp[b,g,k,h+dy,w+dx,c] - krev[g,dy,dx,k,c,f] )
  where ygp[b,g,k] = x[b, (g+k-1) mod 4] spatially padded with +inf and
  krev = the 4 planar rotations of the depth-rotated SE, spatially reversed.

Sharding: core -> (g = core//2, f-half = core%2).  Each core computes all 4
batches for one group-rotation g and 4 of the 8 filters.

Layout "c-block": partition p = 32*c + hs (c = channel, hs = h mod 32), free
dims (hb, b, w) with h = 32*hb + hs.  Each partition sees exactly one channel,
so the per-(tap, f) SE value is a per-partition scalar for FULL-width
tensor_scalar instructions (no chunk splitting).  The channel sum happens on
the host (c pieces live on different partitions).

The 15 (k, dy)-shifted input planes are pre-built on the host and DMA'd as 15
contiguous tiles via HWDGE on the otherwise-idle SP engine (SWDGE descriptor
generation would burn the Pool engine, which now does compute).

The 75 taps x 4 filters of acc = min(acc, window - kk) are split across the
DVE and ACT engines (greedy balance by simulated per-engine cost; the Pool
engine is DMA-only on this walrus backend — its elementwise tensor ops fail
the codegen engine check):
  - DVE-own taps (even dx only, for 4B-aligned 4x packed reads):
      4x tensor_scalar subtract (4x mode) into a private tmp + one full-width
      (4 filters at once) tensor_tensor min (2x mode) into acc_d.
  - ACT-assist taps: 4x activation(Identity, bias=-kk) subs into a ring slot,
      DVE does the full-width min into acc_d (one cross-engine wait; stale
      same-engine hazard waits are stripped post-build so every compute
      instruction fits its single ISA sync-wait slot).
The final tap runs per-filter on DVE so each filter's Pool-sequencer SWDGE
output DMA starts while the remaining filters still compute; the host sums
the channel pieces and reassembles.
"""

import os
from contextlib import ExitStack

import numpy as np

import concourse.bass as bass
import concourse.mybir as mybir
import concourse.tile as tile
from concourse.bass_utils import run_bass_kernel_spmd

B, G, H, W, C = 4, 4, 96, 96, 4
KH, KW, F = 5, 5, 8
PAD = 2
WP = W + PAD * 2  # 100
NTAP = 3 * KH * KW  # 75
N_CORES = 8
NP = 4  # batches per core
NF = F // 2  # filters per core
HS = 32  # h rows per (c, hb) block
HB = 3  # h blocks
BIG = 30000.0  # +inf stand-in that survives fp16

CFG_REPEAT = int(os.environ.get("KCFG_REPEAT", "1"))
# simulated per-unit costs (ns) used by the static scheduler.  The Pool
# engine is DMA-only on this walrus backend (all its elementwise tensor ops
# fail the codegen engine check), so compute is split DVE vs ACT only.
COST_DVE_SUB = 360.0
COST_DVE_MIN_WIDE = 2460.0
# activation + amortized absorber copies; KCFG_ACT_COST recalibrates the
# DVE/ACT split against real hardware ratios (higher -> fewer ACT taps)
COST_ACT_SUB = float(os.environ.get("KCFG_ACT_COST", "1150"))

FP16 = mybir.dt.float16

_prog_cache = {}
LAST_RESULTS = None


def _taps():
    return [(k, dy, dx) for k in range(3) for dy in range(KH) for dx in range(KW)]


def _schedule():
    """Greedy static assignment of the 75 taps to engines.

    Returns list of 'dve' | 'act' | 'pool' per tap index.  DVE-own taps are
    restricted to even dx (aligned 4x packed reads); the first tap must be
    'dve' so acc_dve exists before any assist-min, and the first 'pool' tap
    initializes acc_pool.
    """
    taps = _taps()
    t_dve = t_act = 0.0
    out = []
    for ti, (k, dy, dx) in enumerate(taps):
        # projected end-times if this tap went to each engine; 'act3' is the
        # mixed form (ACT subs 3 filters, DVE subs the 4th + the wide min),
        # only for even dx (the DVE sub needs the aligned 4x window)
        cand = []
        if dx % 2 == 0:
            cand.append(("dve", t_dve + NF * COST_DVE_SUB + COST_DVE_MIN_WIDE))
            for n in (1, 2, 3):
                cand.append((f"act{n}", max(
                    t_act + n * COST_ACT_SUB,
                    t_dve + (NF - n) * COST_DVE_SUB + COST_DVE_MIN_WIDE)))
        cand.append(("act", max(t_act + NF * COST_ACT_SUB,
                                t_dve + COST_DVE_MIN_WIDE)))
        if ti == 0 or ti == len(taps) - 1:
            # tap 0 initializes acc_d; the last tap is processed per-filter
            # on DVE so each filter's output DMA can start early
            cand = [c for c in cand if c[0] == "dve"]
        eng, _ = min(cand, key=lambda c: c[1])
        if eng == "dve":
            t_dve += NF * COST_DVE_SUB + COST_DVE_MIN_WIDE
        elif eng.startswith("act") and eng != "act":
            n = int(eng[3:])
            t_act += n * COST_ACT_SUB
            t_dve += (NF - n) * COST_DVE_SUB + COST_DVE_MIN_WIDE
        else:
            t_act += NF * COST_ACT_SUB
            t_dve += COST_DVE_MIN_WIDE
        out.append(eng)
    return out


class _SplitDrainTC(tile.TileContext):
    """TileContext whose kernel-tail drain is split into one drain per sem
    lane: the stock single Drain carries a wait for every lane used, which
    overflows the CTRL struct's sync-wait encoding on this compiler."""

    def _drain_and_barrier(self, tick_clock, wait_clock):
        from concourse.tile_sem_assignment import N_PROCS
        from concourse.vector_clock import ScopedClock, VectorClock

        gc = tick_clock.global_clock
        ticks = [gc[p] for p in range(N_PROCS)]
        for p in range(N_PROCS):
            if ticks[p] <= 0:
                continue
            sub = [ticks[q] if q == p else 0 for q in range(N_PROCS)]
            d = self.nc.sync.drain()
            wait_clock.add_sem_waits(d.ins, ScopedClock({None: VectorClock(sub)}))

        self.nc.all_engine_barrier()
        assert self.sems is not None
        popped = self.nc._tile_sem_poison_stack.pop()
        assert popped is self._sem_poison
        self.nc.clear_and_free_semaphores(list(self.sems.allocated().values()))
        self.nc.all_engine_barrier()


def _strip_stale_same_engine_waits(nc, lag=8):
    """Drop same-engine sem waits whose producer finished >= `lag` own-engine
    instructions earlier.

    This tile version emits a sem wait for EVERY hazard, including same-engine
    WAW/WAR whose producers are long retired; compute ISA structs can encode
    only ONE sync wait, so a ring-buffer rewrite (same-engine WAW + cross-
    engine WAR) overflows codegen.  Engines issue in order and their writes
    land within a couple of instructions, so a same-engine wait on a producer
    `lag` instructions back is vacuous.  Recent same-engine waits (pipelined
    RAW guards) are kept.
    """
    strip_types = {
        "InstActivation", "InstTensorScalarPtr", "InstTensorTensor",
        "InstTensorScalar", "InstMemset", "InstCopy", "InstTensorCopy",
        "InstTensorReduce",
    }
    counts = {}
    fn = nc.m.functions[0]
    for blk in fn.blocks:
        for ins in blk.instructions:
            si = ins.sync_info
            if si is None:
                continue
            eng = getattr(ins, "engine", None)
            ename = getattr(eng, "name", None) or (str(eng).split(".")[-1] if eng else "")
            if si.on_wait and type(ins).__name__ in strip_types and ename in (
                "Activation", "DVE", "Pool", "PE"
            ):
                keep = []
                for w in si.on_wait:
                    nm = w.ant_name or ""
                    if (
                        nm.startswith(ename + "_")
                        and w.wait_mode == "sem-ge-imm"
                        and w.wait_value is not None
                        and counts.get(nm, 0) - w.wait_value >= lag
                    ):
                        continue
                    keep.append(w)
                if len(keep) != len(si.on_wait):
                    si.on_wait = keep
            for u in si.on_update or []:
                if u.ant_name:
                    counts[u.ant_name] = counts.get(u.ant_name, 0) + (u.update_value or 1)
    return nc


def _build_program(repeat=1):
    import concourse.tile_sem_assignment as _tsa

    _orig_swdge = _tsa.NUM_SWDGE_GLOBAL_SEMS
    _tsa.NUM_SWDGE_GLOBAL_SEMS = 4
    try:
        return _strip_stale_same_engine_waits(_build_program_inner(repeat))
    finally:
        _tsa.NUM_SWDGE_GLOBAL_SEMS = _orig_swdge


def _build_program_inner(repeat=1):
    nc = bass.Bass()
    sched = _schedule()
    taps = _taps()

    # xin2[t15][p][hb][b][wp]: host-pre-shifted planes, one contiguous tile per
    # (k, dy).  kk: +kk columns then -kk columns (ACT bias), col = ti*NF + fi.
    xin = nc.declare_dram_parameter("xin", [15, 128, HB, NP, WP], FP16, isOutput=False)
    kkin = nc.declare_dram_parameter("kk", [128, 2 * NTAP * NF], mybir.dt.float32, isOutput=False)
    yout = nc.declare_dram_parameter("yout", [128, NF, HB, NP, W], FP16, isOutput=True)

    with _SplitDrainTC(nc) as tc, ExitStack() as ctx:
        pool = ctx.enter_context(tc.tile_pool(name="main", bufs=1))

        # Compute-instruction ISA slots can encode only ONE sync wait, so
        # "touch" every DMA'd region with a trivial op on each consuming
        # engine right after its DMA; later compute instructions inherit the
        # dependency through engine program order and carry no waits.
        touch_v = pool.tile([1, 256], mybir.dt.float32, name="touch_v", tag="touch_v")
        touch_s = pool.tile([1, 256], mybir.dt.float32, name="touch_s", tag="touch_s")
        touch_g = pool.tile([1, 256], mybir.dt.float32, name="touch_g", tag="touch_g")
        tctr = [0, 0, 0]

        def _touch(src, engines):
            # columns cycle mod 256: the WAW producer of a reused column is
            # hundreds of instructions back, so its same-engine wait is
            # stripped by _strip_stale_same_engine_waits.
            if "v" in engines:
                tctr[0] += 1
                i = tctr[0] % 256
                nc.vector.tensor_scalar_add(touch_v[0:1, i : i + 1], src, 0.0)
            if "s" in engines:
                tctr[1] += 1
                i = tctr[1] % 256
                nc.scalar.copy(touch_s[0:1, i : i + 1], src)
            if "g" in engines:
                tctr[2] += 1
                i = tctr[2] % 256
                nc.gpsimd.tensor_scalar_add(touch_g[0:1, i : i + 1], src, 0.0)

        # which engines read each (k, dy) tile
        tile_readers = {}
        for ti, (k, dy, dx) in enumerate(taps):
            e = sched[ti]
            rd = {"v"} if e == "dve" else ({"s"} if e == "act" else {"s", "v"})
            tile_readers.setdefault((k, dy), set()).update(rd)

        # kkt first: every tap's first instruction reads it, so it must land
        # before any compute can start.
        kkt = pool.tile([128, 2 * NTAP * NF], mybir.dt.float32, name="kkt", tag="kkt")
        nc.sync.dma_start(kkt[:], kkin[:])
        _touch(kkt[0:1, 0:1], {"v", "s", "g"})

        in_t = {}
        for k in range(3):
            for dy in range(KH):
                t = pool.tile([128, HB, NP, WP], FP16, name=f"in_{k}_{dy}", tag=f"in_{k}_{dy}")
                nc.sync.dma_start(t[:], xin[(k * KH + dy)])
                _touch(t[0:1, 0, 0, 0:1], tile_readers[(k, dy)])
                in_t[k, dy] = t

        yout_f = [yout[:, fi] for fi in range(NF)]
        acc_d = pool.tile([128, NF, HB, NP, W], FP16, name="acc_d", tag="acc_d")
        tmp_d = pool.tile([128, NF, HB, NP, W], FP16, name="tmp_d", tag="tmp_d")
        NRING = 6
        ring = [
            pool.tile([128, NF, HB, NP, W], FP16, name=f"ring_{i}", tag=f"ring_{i}")
            for i in range(NRING)
        ]

        ring_i = 0
        for _rep in range(repeat):
            for ti, (k, dy, dx) in enumerate(taps):
                eng = sched[ti]
                src = in_t[k, dy]
                win = src[:, :, :, dx : dx + W]
                if eng == "dve":
                    last = ti == len(taps) - 1 and _rep == repeat - 1
                    dst = acc_d if ti == 0 else tmp_d
                    for fi in range(NF):
                        kk_ap = kkt[:, ti * NF + fi : ti * NF + fi + 1]
                        nc.vector.tensor_scalar(
                            dst[:, fi], win, kk_ap, None, mybir.AluOpType.subtract
                        )
                        if last:
                            # per-filter finish: min this filter now and ship
                            # it while the remaining filters still compute
                            nc.vector.tensor_tensor(
                                acc_d[:, fi], tmp_d[:, fi], acc_d[:, fi],
                                mybir.AluOpType.min,
                            )
                            _touch(acc_d[0:1, fi, 0, 0, 0:1], {"g"})
                            nc.gpsimd.dma_start(yout_f[fi], acc_d[:, fi])
                    if ti != 0 and not last:
                        nc.vector.tensor_tensor(
                            acc_d[:], tmp_d[:], acc_d[:], mybir.AluOpType.min
                        )
                else:
                    # 'act' (ACT subs all 4 filters) or 'actN' (ACT subs N,
                    # DVE subs the rest into the same slot).  Ring-rewrite
                    # hazards: the same-engine WAW wait is stripped post-build
                    # (stale), leaving only the WAR wait on the DVE min that
                    # read this slot -- one sem, fits.
                    slot = ring[ring_i % NRING]
                    ring_i += 1
                    n_act_f = NF if eng == "act" else int(eng[3:])
                    for fi in range(n_act_f):
                        negkk_ap = kkt[:, NTAP * NF + ti * NF + fi : NTAP * NF + ti * NF + fi + 1]
                        nc.scalar.activation(
                            slot[:, fi], win,
                            mybir.ActivationFunctionType.Identity, bias=negkk_ap,
                        )
                    # DVE absorber on ACT's last sub: the wide min then
                    # inherits all ACT slice deps through DVE program order
                    # (TT structs encode only one sync wait).
                    _touch(slot[0:1, n_act_f - 1, 0, 0, 0:1], {"v"})
                    for fi in range(n_act_f, NF):
                        # DVE subs the remaining filters into the same slot
                        # (same-engine dep for the min, program order)
                        kk_ap = kkt[:, ti * NF + fi : ti * NF + fi + 1]
                        nc.vector.tensor_scalar(
                            slot[:, fi], win, kk_ap, None, mybir.AluOpType.subtract
                        )
                    nc.vector.tensor_tensor(
                        acc_d[:], slot[:], acc_d[:], mybir.AluOpType.min
                    )



    return nc


def _get_program(repeat=1):
    key = repeat
    if key not in _prog_cache:
        _prog_cache[key] = _build_program(repeat)
    return _prog_cache[key]


def _krev(kernel):
    """[g, dy, dx, k, c, f] rotated/reversed SE, pure re-indexing of `kernel`."""
    k_ero = np.stack(
        [
            np.rot90(kernel[:, :, 2], k=3, axes=(0, 1)),
            kernel[:, :, 1],
            np.rot90(kernel[:, :, 0], k=1, axes=(0, 1)),
        ],
        axis=2,
    )
    krot = np.stack([np.rot90(k_ero, k=j, axes=(0, 1)) for j in range(4)], axis=0)
    return krot[:, ::-1, ::-1]


def _core_units(core):
    g = core // 2
    fh = core % 2
    return g, list(range(B)), list(range(fh * NF, fh * NF + NF))


def _make_in_map(x, kr, core):
    g, bs, fs = _core_units(core)
    # padded planes ygp[k][b, c, h', w'] (h', w' in [0, 100))
    xin = np.empty((15, 128, HB, NP, WP), np.float16)
    for k in range(3):
        src = x[:, (g + k - 1) % 4]  # [B, H, W, C]
        ygp = np.full((NP, C, H + 2 * PAD, WP), BIG, np.float32)
        for bi, b in enumerate(bs):
            ygp[bi, :, PAD : PAD + H, PAD : PAD + W] = src[b].transpose(2, 0, 1)
        for dy in range(KH):
            # tile[p=(c,hs), hb, b, wp] = ygp[b, c, hb*32+hs+dy, wp]
            v = ygp[:, :, dy : dy + H, :]  # [b, c, 96, 100]
            v = v.reshape(NP, C, HB, HS, WP)  # [b, c, hb, hs, wp]
            v = v.transpose(1, 3, 2, 0, 4)  # [c, hs, hb, b, wp]
            xin[k * KH + dy] = v.reshape(128, HB, NP, WP).astype(np.float16)
    # kk columns: +kk then -kk, col = ti*NF + fi, value kr[g, dy, dx, k, c(p), f]
    sel = kr[g][:, :, :, :, fs]  # [dy, dx, k, c, NF]
    tap_cf = np.ascontiguousarray(sel.transpose(2, 0, 1, 3, 4)).reshape(NTAP, C, NF)
    kk = np.empty((128, 2 * NTAP * NF), np.float32)
    for c in range(C):
        block = tap_cf[:, c, :].reshape(NTAP * NF)  # [ti*NF+fi]
        kk[c * HS : (c + 1) * HS, :NTAP * NF] = block[None, :]
        kk[c * HS : (c + 1) * HS, NTAP * NF :] = -block[None, :]
    return {"xin": xin, "kk": kk}


def _assemble(results):
    out = np.zeros((B, G, H, W, F), np.float32)
    for core in range(N_CORES):
        g, bs, fs = _core_units(core)
        y = np.asarray(results[core]["yout"]).astype(np.float32)
        # y[p=(c,hs), fi, hb, b, w] -> sum over c -> out[b, g, 32*hb+hs, w, f]
        y = y.reshape(C, HS, NF, HB, NP, W).sum(axis=0)  # [hs, fi, hb, b, w]
        y = y.transpose(3, 2, 0, 4, 1)  # [b, hb, hs, w, fi]
        y = y.reshape(NP, H, W, NF)
        for bi, b in enumerate(bs):
            out[b, g, :, :, fs[0] : fs[0] + NF] = y[bi]
    return out


def kernel(x, kernel):
    x = np.ascontiguousarray(np.asarray(x, dtype=np.float32))
    se = np.ascontiguousarray(np.asarray(kernel, dtype=np.float32))
    kr = _krev(se)  # [g, dy, dx, k, c, f]
    in_maps = [_make_in_map(x, kr, core) for core in range(N_CORES)]
    nc = _get_program(CFG_REPEAT)
    res = run_bass_kernel_spmd(nc, in_maps, list(range(N_CORES)), trace=False)
    global LAST_RESULTS
    LAST_RESULTS = res
    return _assemble(res.results)
